# revision 32
# baseline (speedup 1.0000x reference)
"""Trainium2 Bass kernel for the Koopman control-model chain.

Computes, for fixed-size inputs L[4096,4096], R[2048,2048], B[2048,256]:
    M   = L @ L.T            (blocks M11, M21, M22 only)
    F   = M21, P = M22
    E   = (M11 + M22)/2 + (R - R.T)/2
    Acl = solve(E, F)        (block LU without pivoting, Newton-inverted
                              128x128 diagonal blocks)
    A   = (I - B @ (B.T @ P)) @ Acl

Distribution: 8 NeuronCores, column-sharded (each core owns a 256-column
slice of every 2048-wide intermediate).  The Gram phase and the triangular
substitutions are column-parallel; E's factorization is replicated on all
cores after an AllGather of S = (M11+M22)/2 + skew.  A second small
AllGather distributes U1 = P @ B for the output chain.

Perf notes vs the first working version:
  - All DRAM inputs are host-pre-tiled so every big DMA moves long
    contiguous lines (16 KB per partition) instead of 512 B gathers.
  - The h=1 Gram pass computes [M21 | M22] with a fused 512-wide moving
    operand (one weight load per k-tile instead of two).
  - Scale factors (0.5 on M11/M22, 2 on B) are folded into the host-side
    input preparation, removing the on-device 0.5* scaling pass.
  - S is AllGathered in two halves with Shared outputs; the second half
    plus the U1 AllGather overlap the tail of the Gram phase.
  - Newton-Schulz iterations use the 2I-DX form (3 matmuls, 1 DVE op,
    2 copies with one on the scalar engine) instead of 3 matmuls + 5 DVE.
  - A short warm-up matmul burst keeps the PE HAM clock-gate open while
    the initial input DMAs stream.

All matmuls run in float32r; accumulation is fp32 in PSUM.
"""

import ml_dtypes
import numpy as np

import concourse.bass as bass  # noqa: F401  (registers engines)
import concourse.mybir as mybir
import concourse.tile as tile
from concourse import bacc
from concourse.bass_utils import run_bass_kernel_spmd

F32 = mybir.dt.float32
F32R = mybir.dt.float32r
BF16 = mybir.dt.bfloat16
P = 128

LAST_EXEC_NS = None


def round_f32r(x: np.ndarray) -> np.ndarray:
    """Round fp32 to the PE's fp32r input format (RNE to 11 mantissa bits)."""
    u = np.ascontiguousarray(x, np.float32).view(np.uint32)
    r = ((u.astype(np.uint64) + ((u.astype(np.uint64) >> 12) & 1) + 0x7FF)
         & ~np.uint64(0xFFF)).astype(np.uint32)
    return r.view(np.float32)


class Cfg:
    def __init__(self, d=2048, v=256, ncores=8, newton0=26, newton=10,
                 warm=256):
        self.d = d                    # dim_K
        self.v = v                    # B's column count
        self.ncores = ncores
        self.kdim = 2 * d             # Gram contraction length (rows of L.T)
        self.cw = d // ncores         # per-core column slice
        assert self.cw == 256, "strip width must equal per-core slice (256)"
        self.nb = d // P              # 128-blocks along d
        self.kt = self.kdim // P      # k-tiles in the Gram contraction
        self.ns = self.nb // 2        # 256-wide strips
        self.nv = v // P              # 128-blocks along v
        self.newton = [newton0] + [newton] * (self.nb - 1)
        self.warm = warm


def build_program(cfg: Cfg):
    d, v, cw, nb, kt, ns, nv = (cfg.d, cfg.v, cfg.cw, cfg.nb, cfg.kt,
                                cfg.ns, cfg.nv)
    nc = bacc.Bacc("TRN2", target_bir_lowering=False, debug=False,
                   num_devices=cfg.ncores)

    # Pre-tiled inputs (see run() for the host-side layout):
    #   lt_t[m*128+p, t*128+q] = LT[t*128+p, colbase(m)+q], h=0 tiles * 0.5
    lt_in = nc.dram_tensor("lt", [cfg.kdim, cfg.kdim], BF16,
                           kind="ExternalInput").ap()
    rhs_in = nc.dram_tensor("rhs", [P, kt * 2 * cw], BF16,
                            kind="ExternalInput").ap()
    b_in = nc.dram_tensor("b", [P, nb * v], F32R, kind="ExternalInput").ap()
    bt_in = nc.dram_tensor("bt", [P, nv * d], F32R, kind="ExternalInput").ap()
    sk_in = nc.dram_tensor("sk", [P, nb * cw], F32, kind="ExternalInput").ap()
    iden_in = nc.dram_tensor("iden", [P, P], F32, kind="ExternalInput").ap()
    iden2_in = nc.dram_tensor("iden2", [P, P], F32, kind="ExternalInput").ap()
    ones_in = nc.dram_tensor("onesv", [P, 1], F32, kind="ExternalInput").ap()
    onesr_in = nc.dram_tensor("onesr", [1, P], F32, kind="ExternalInput").ap()
    a_out = nc.dram_tensor("a_out", [P, nb * cw], F32,
                           kind="ExternalOutput").ap()

    rg = [list(range(cfg.ncores))]
    sub = mybir.AluOpType.subtract
    add = mybir.AluOpType.add
    mult = mybir.AluOpType.mult
    CopyFn = mybir.ActivationFunctionType.Copy
    half = nb // 2  # 8 row-blocks per S AllGather half

    with tile.TileContext(nc) as tc:
        with (
            tc.tile_pool(name="const", bufs=1) as const,
            tc.tile_pool(name="flong", bufs=1) as flong,
            tc.tile_pool(name="dram", bufs=1, space="DRAM") as dram,
        ):
            iden = const.tile([P, P], F32, tag="iden")
            iden2 = const.tile([P, P], F32, tag="iden2")
            ones_c = const.tile([P, 1], F32, tag="ones_c")
            ones_r = const.tile([1, P], F32, tag="ones_r")
            nc.sync.dma_start(iden[:], iden_in[:])
            nc.sync.dma_start(iden2[:], iden2_in[:])
            nc.sync.dma_start(ones_c[:], ones_in[:])
            nc.sync.dma_start(ones_r[:], onesr_in[:])

            # F-tiles (later Y, then X): one [128, cw] f32r tile per row-block
            fY = [flong.tile([P, cw], F32R, tag=f"fY{i}", name=f"fY{i}")
                  for i in range(nb)]

            # S AllGather quarters: shard layout [(p t), n] with t the local
            # row-block index, so factorization strip staging is one long
            # contiguous DMA per quarter.  Quarter q is kicked as soon as
            # row-blocks 4q..4q+3 of S are assembled, overlapping the
            # collectives with the remaining Gram work.
            QB = nb // 4  # row-blocks per AllGather quarter
            sag_b = [dram.tile([P * QB, cw], F32, name=f"sagb{q}")
                     for q in range(4)]
            sagg = [dram.tile([cfg.ncores * P * QB, cw], F32,
                              addr_space="Shared", name=f"sagg{q}")
                    for q in range(4)]
            u1_b = dram.tile([cw, v], F32R)
            u1ag = dram.tile([cfg.ncores * cw, v], F32R, addr_space="Shared")
            sink = dram.tile([1, 1], F32)
            # Tiny dummy AllGather issued first: absorbs the one-time
            # collective-stack init (~40 us) during the DMA-bound startup
            # so the first real S quarter isn't delayed by it.
            dumb = dram.tile([8, 8], F32)
            dumg = dram.tile([cfg.ncores * 8, 8], F32, addr_space="Shared")

            # ---------------- Phase A: Gram slices ----------------
            with (
                tc.tile_pool(name="gram", bufs=1) as gram,
                tc.tile_pool(name="slabs", bufs=4) as slabs,
                tc.tile_pool(name="gsm", bufs=4) as gsm,
                tc.tile_pool(name="psA", bufs=2, space="PSUM") as psA,
                tc.tile_pool(name="psW", bufs=1, space="PSUM") as psW,
            ):
                nc.gpsimd.collective_compute(
                    "AllGather", mybir.AluOpType.bypass,
                    ins=[dumb.opt()], outs=[dumg.opt()], replica_groups=rg)
                # PE warm-up: keep the HAM clock-gate open while the first
                # input DMAs stream.  One long accumulation group so DCE
                # keeps every matmul; a 1-elem sink DMA anchors the result.
                ps_w = psW.tile([P, P], F32, tag="warm")
                for w in range(cfg.warm):
                    nc.tensor.matmul(ps_w[:], iden[:], iden[:],
                                     start=(w == 0), stop=(w == cfg.warm - 1))
                w_sb = gsm.tile([1, 1], F32, tag="wsb")
                nc.vector.tensor_copy(w_sb[:], ps_w[0:1, 0:1])
                nc.sync.dma_start(sink[:], w_sb[:])

                rhs = gram.tile([P, kt * 2 * cw], BF16, tag="rhs")
                nc.sync.dma_start(rhs[:], rhs_in[:])
                b_sb = gram.tile([P, nb * v], F32R, tag="b_sb")
                nc.sync.dma_start(b_sb[:], b_in[:])
                sk_sb = gram.tile([P, nb * cw], F32, tag="sk_sb")
                nc.sync.dma_start(sk_sb[:], sk_in[:])

                m22 = gram.tile([P, nb * cw], F32R, tag="m22")
                s_t = gram.tile([P, nb * cw], F32, tag="s_t")

                for m in range(nb):
                    # h=1 slab (tile 16+m): fused [M21 | 0.5*M22] pass
                    slab1 = slabs.tile([P, kt * P], BF16, tag="slab")
                    nc.sync.dma_start(slab1[:],
                                      lt_in[(nb + m) * P:(nb + m + 1) * P, :])
                    # h=0 slab (tile m, pre-scaled 0.5): M11 pass
                    slab0 = slabs.tile([P, kt * P], BF16, tag="slab")
                    nc.sync.dma_start(slab0[:], lt_in[m * P:(m + 1) * P, :])

                    ps = psA.tile([P, 2 * cw], F32, tag="gps")
                    for t in range(kt):
                        nc.tensor.matmul(ps[:], slab1[:, t * P:(t + 1) * P],
                                         rhs[:, t * 2 * cw:(t + 1) * 2 * cw],
                                         start=(t == 0), stop=(t == kt - 1))
                    nc.vector.tensor_copy(fY[m][:], ps[:, 0:cw])
                    nc.vector.tensor_copy(m22[:, m * cw:(m + 1) * cw],
                                          ps[:, cw:2 * cw])

                    ps2 = psA.tile([P, cw], F32, tag="gps2")
                    for t in range(kt):
                        nc.tensor.matmul(ps2[:], slab0[:, t * P:(t + 1) * P],
                                         rhs[:, t * 2 * cw:t * 2 * cw + cw],
                                         start=(t == 0), stop=(t == kt - 1))
                    msl = slice(m * cw, (m + 1) * cw)
                    t1 = gsm.tile([P, cw], F32, tag="t1")
                    nc.vector.tensor_tensor(t1[:], ps2[:],
                                            m22[:, msl].bitcast(F32), op=add)
                    nc.vector.tensor_tensor(s_t[:, msl], t1[:], sk_sb[:, msl],
                                            op=add)

                    if m % QB == QB - 1:
                        q = m // QB
                        nc.gpsimd.dma_start(
                            sag_b[q].rearrange("(p t) n -> p t n", t=QB),
                            s_t[:, q * QB * cw:(q + 1) * QB * cw]
                            .rearrange("p (t n) -> p t n", n=cw))
                        nc.gpsimd.collective_compute(
                            "AllGather", mybir.AluOpType.bypass,
                            ins=[sag_b[q].opt()], outs=[sagg[q].opt()],
                            replica_groups=rg)

                # U1_c = (P @ B)[c-rows] : lhsT = m22 column slices (holds
                # 0.5*M22; b_sb holds 2*B, so the product is M22 @ B).
                u1s = gram.tile([P, (cw // P) * v], F32R, tag="u1s")
                for mh in range(cw // P):
                    psu = psA.tile([P, v], F32, tag="gps2")
                    for k in range(nb):
                        nc.tensor.matmul(
                            psu[:],
                            m22[:, k * cw + mh * P:k * cw + (mh + 1) * P],
                            b_sb[:, k * v:(k + 1) * v],
                            start=(k == 0), stop=(k == nb - 1))
                    nc.vector.tensor_copy(u1s[:, mh * v:(mh + 1) * v], psu[:])
                nc.gpsimd.dma_start(
                    u1_b[:].rearrange("(t p) n -> p t n", p=P),
                    u1s[:].rearrange("p (t n) -> p t n", n=v))
                nc.gpsimd.collective_compute(
                    "AllGather", mybir.AluOpType.bypass,
                    ins=[u1_b.opt()], outs=[u1ag.opt()], replica_groups=rg)

            # ------------- Phase B: replicated factorization -------------
            with tc.tile_pool(name="epool", bufs=1) as epool:
                es = [[epool.tile([P, 2 * P], F32R, tag=f"e{s}_{i}",
                                 name=f"e{s}_{i}")
                       for i in range(nb)] for s in range(ns)]
                wT = [epool.tile([P, P], F32R, tag=f"wT{j}", name=f"wT{j}")
                      for j in range(nb)]
                wN = [epool.tile([P, P], F32R, tag=f"wN{j}", name=f"wN{j}")
                      for j in range(nb)]

                with (
                    tc.tile_pool(name="work", bufs=2) as work,
                    tc.tile_pool(name="nwt", bufs=2) as nwt,
                    tc.tile_pool(name="stg", bufs=4) as stg,
                    tc.tile_pool(name="psB", bufs=3, space="PSUM") as psB,
                    tc.tile_pool(name="psS", bufs=4, space="PSUM") as psS,
                ):
                    def pe_transpose(src_ap):
                        """128x128 transpose via PE; returns an f32r SBUF tile."""
                        pst = psS.tile([P, P], F32, tag="sps")
                        nc.tensor.transpose(pst[:], src_ap.bitcast(F32), iden[:])
                        out = stg.tile([P, P], F32R, tag="tps", bufs=2)
                        nc.vector.tensor_copy(out[:], pst[:])
                        return out

                    def etile(i, k):
                        """[128,128] slice of E-storage at block (i, k)."""
                        return es[k // 2][i][:, (k % 2) * P:(k % 2 + 1) * P]

                    def newton(j, d_n):
                        """Invert D_j; writes wT[j] (=W^T) and wN[j] (=W).

                        Bulk Newton-Schulz iterations run in bf16 (f32r
                        matmuls with a 128-wide moving operand cost 4
                        cycles/row on the PE; bf16 costs 1).  Newton-Schulz
                        is self-correcting, so two final f32r polish
                        iterations restore full f32r accuracy.
                        """
                        dT = pe_transpose(d_n)
                        # alpha = 1 / sum(D*D)
                        sq = stg.tile([P, P], F32, tag="sq", bufs=2)
                        nc.vector.tensor_tensor(sq[:], d_n.bitcast(F32),
                                                d_n.bitcast(F32), op=mult)
                        rowsum = stg.tile([P, 1], F32, tag="rsum")
                        nc.vector.tensor_reduce(rowsum[:], sq[:],
                                                axis=mybir.AxisListType.X,
                                                op=add)
                        pss = psS.tile([1, 1], F32, tag="sps")
                        nc.tensor.matmul(pss[:], rowsum[:], ones_c[:],
                                         start=True, stop=True)
                        alph = stg.tile([1, 1], F32, tag="alph")
                        nc.vector.reciprocal(alph[:], pss[:])
                        psb = psS.tile([P, 1], F32, tag="sps")
                        nc.tensor.matmul(psb[:], ones_r[:], alph[:],
                                         start=True, stop=True)
                        ab = stg.tile([P, 1], F32, tag="ab")
                        nc.vector.tensor_copy(ab[:], psb[:])
                        # X0 = alpha D^T (xN), X0^T = alpha D (y)
                        xN = nwt.tile([P, P], F32R, tag="xN")
                        nc.vector.tensor_scalar_mul(xN[:], dT[:].bitcast(F32),
                                                    ab[:])
                        y = nwt.tile([P, P], F32R, tag="y")
                        nc.vector.tensor_scalar_mul(y[:], d_n.bitcast(F32),
                                                    ab[:])
                        # Newton-Schulz in 2I-DX form:
                        #   Z2 = 2I - D X;  X' = X Z2;  X'^T = Z2^T X^T
                        for it in range(cfg.newton[j]):
                            last = (it == cfg.newton[j] - 1)
                            psz = psS.tile([P, P], F32, tag="sps")
                            nc.tensor.matmul(psz[:], dT[:], xN[:],
                                             start=True, stop=True)
                            z2 = stg.tile([P, P], F32R, tag="z2f", bufs=2)
                            nc.vector.tensor_tensor(z2[:], iden2[:], psz[:],
                                                    op=sub)
                            psp = psS.tile([P, P], F32, tag="sps")
                            nc.tensor.matmul(psp[:], y[:], z2[:],
                                             start=True, stop=True)
                            pspt = psS.tile([P, P], F32, tag="sps")
                            nc.tensor.matmul(pspt[:], z2[:], y[:],
                                             start=True, stop=True)
                            xN2 = wN[j] if last else nwt.tile([P, P], F32R,
                                                              tag="xN")
                            nc.vector.tensor_copy(xN2[:], psp[:])
                            y2 = wT[j] if last else nwt.tile([P, P], F32R,
                                                             tag="y")
                            nc.vector.tensor_copy(y2[:], pspt[:])
                            xN, y = xN2, y2

                    def fwd_sub(j):
                        """Forward substitution on the local F slice."""
                        psf = psB.tile([P, cw], F32, tag="bps")
                        for k in range(j):
                            nc.tensor.matmul(psf[:], etile(j, k), fY[k][:],
                                             start=(k == 0),
                                             stop=(k == j - 1))
                        nc.vector.tensor_tensor(fY[j][:],
                                                fY[j][:].bitcast(F32),
                                                psf[:], op=sub)

                    stages = {}
                    def stage_load(s):
                        """Load strip s of S from the gathered quarters."""
                        qt = []
                        for q in range(4):
                            st = work.tile([P, QB * 2 * P], F32,
                                           tag=f"stq{q}", name=f"stq{q}")
                            nc.sync.dma_start(
                                st[:].rearrange("p (t n) -> p t n", n=2 * P),
                                sagg[q][s * QB * P:(s + 1) * QB * P, :]
                                .rearrange("(p t) n -> p t n", t=QB))
                            qt.append(st)
                        stages[s] = qt

                    def stage_slice(s, i):
                        st = stages[s][i // QB]
                        i = i % QB
                        return st[:, i * 2 * P:(i + 1) * 2 * P]

                    def panels(j, i0, i1):
                        """L_ij^T = W^T tmp^T for i in [i0, i1), in place.

                        Pairs of rows share the W^T weight load and fuse to
                        one 256-wide matmul (f32r at 1 cycle/row instead of
                        the 4 cycles/row a 128-wide moving operand costs).
                        """
                        i = i0
                        while i < i1:
                            if i + 1 < i1:
                                pst1 = psS.tile([P, P], F32, tag="sps")
                                nc.tensor.transpose(
                                    pst1[:], etile(i, j).bitcast(F32), iden[:])
                                pst2 = psS.tile([P, P], F32, tag="sps")
                                nc.tensor.transpose(
                                    pst2[:], etile(i + 1, j).bitcast(F32),
                                    iden[:])
                                tp2 = stg.tile([P, 2 * P], F32R, tag="tp2", bufs=2)
                                nc.vector.tensor_copy(tp2[:, 0:P], pst1[:])
                                nc.vector.tensor_copy(tp2[:, P:2 * P], pst2[:])
                                psl = psS.tile([P, 2 * P], F32, tag="sps")
                                nc.tensor.matmul(psl[:], wN[j][:], tp2[:],
                                                 start=True, stop=True)
                                nc.vector.tensor_copy(etile(i, j),
                                                      psl[:, 0:P])
                                nc.vector.tensor_copy(etile(i + 1, j),
                                                      psl[:, P:2 * P])
                                i += 2
                            else:
                                tpt = pe_transpose(etile(i, j))
                                psl = psS.tile([P, P], F32, tag="sps")
                                nc.tensor.matmul(psl[:], wN[j][:], tpt[:],
                                                 start=True, stop=True)
                                nc.vector.tensor_copy(etile(i, j), psl[:])
                                i += 1

                    def odd_term(j, i):
                        """Apply the k=j-1 Schur term to es[s][i] right half."""
                        s = j // 2
                        pst = psB.tile([P, P], F32, tag="bps")
                        nc.tensor.matmul(pst[:], etile(i, j - 1),
                                         es[s][j - 1][:, P:2 * P],
                                         start=True, stop=True)
                        rh = es[s][i][:, P:2 * P]
                        nc.vector.tensor_tensor(rh, rh.bitcast(F32), pst[:],
                                                op=sub)

                    stage_load(0)
                    stage_load(1)

                    # Strips 0/1 are special-cased around the S-AllGather
                    # quarters: work is emitted quarter by quarter in row
                    # order, so newton(0)/newton(1) and the early panels run
                    # while the later S quarters are still being gathered.
                    for i in range(QB):
                        nc.vector.tensor_copy(es[0][i][:], stage_slice(0, i))
                    newton(0, etile(0, 0))
                    panels(0, 1, QB)
                    for i in range(1, QB):
                        odd_term(1, i)
                    fwd_sub(1)
                    newton(1, etile(1, 1))
                    panels(1, 2, QB)
                    for q in range(1, 4):
                        for i in range(q * QB, (q + 1) * QB):
                            nc.vector.tensor_copy(es[0][i][:],
                                                  stage_slice(0, i))
                        panels(0, q * QB, (q + 1) * QB)
                        for i in range(q * QB, (q + 1) * QB):
                            odd_term(1, i)
                        panels(1, q * QB, (q + 1) * QB)
                    blk = es[0][0][:, P:2 * P]
                    tps = pe_transpose(blk)
                    nc.vector.tensor_copy(blk, tps[:])

                    for j in range(2, nb):
                        s, par = j // 2, j % 2
                        if par == 0:
                            # strip entry: left-looking update
                            for i in range(nb):
                                kmax = min(i, j)
                                if kmax == 0:
                                    nc.vector.tensor_copy(es[s][i][:],
                                                          stage_slice(s, i))
                                else:
                                    pst = psB.tile([P, 2 * P], F32,
                                                   tag="bps")
                                    for k in range(kmax):
                                        nc.tensor.matmul(
                                            pst[:], etile(i, k), es[s][k][:],
                                            start=(k == 0),
                                            stop=(k == kmax - 1))
                                    nc.vector.tensor_tensor(
                                        es[s][i][:], stage_slice(s, i),
                                        pst[:], op=sub)
                            if s + 1 < ns:
                                stage_load(s + 1)
                            # rows above the diagonal strip are now final U
                            # blocks: transpose them in place (eagerly, off
                            # the back-substitution critical path).  After
                            # this, etile(i, 2s)/etile(i, 2s+1) for i < 2s
                            # hold U^T, which is exactly the lhsT the
                            # back-substitution matmul needs.
                            for i in range(j):
                                for kk in range(2):
                                    blk = es[s][i][:, kk * P:(kk + 1) * P]
                                    tps = pe_transpose(blk)
                                    nc.vector.tensor_copy(blk, tps[:])
                            if j > 0:
                                fwd_sub(j)
                        else:
                            # odd step: apply the k=j-1 term to right half
                            for i in range(j, nb):
                                pst = psB.tile([P, P], F32, tag="bps")
                                nc.tensor.matmul(pst[:], etile(i, j - 1),
                                                 es[s][j - 1][:, P:2 * P],
                                                 start=True, stop=True)
                                rh = es[s][i][:, P:2 * P]
                                nc.vector.tensor_tensor(
                                    rh, rh.bitcast(F32), pst[:], op=sub)
                            # the superdiagonal U(2s, 2s+1) has now been
                            # consumed as a moving operand; transpose it in
                            # place for the back substitution
                            blk = es[s][j - 1][:, P:2 * P]
                            tps = pe_transpose(blk)
                            nc.vector.tensor_copy(blk, tps[:])
                            fwd_sub(j)

                        newton(j, etile(j, j))
                        panels(j, j + 1, nb)

                    # back substitution (X overwrites fY); etile(j, k) for
                    # k > j already holds U^T from the eager transposes.
                    for j in range(nb - 1, -1, -1):
                        if j < nb - 1:
                            psz = psB.tile([P, cw], F32, tag="bps")
                            for k in range(j + 1, nb):
                                nc.tensor.matmul(psz[:], etile(j, k),
                                                 fY[k][:],
                                                 start=(k == j + 1),
                                                 stop=(k == nb - 1))
                            z = stg.tile([P, cw], F32R, tag="z", bufs=2)
                            nc.vector.tensor_tensor(z[:],
                                                    fY[j][:].bitcast(F32),
                                                    psz[:], op=sub)
                        else:
                            z = fY[j]
                        psx = psB.tile([P, cw], F32, tag="bps")
                        nc.tensor.matmul(psx[:], wT[j][:], z[:],
                                         start=True, stop=True)
                        nc.vector.tensor_copy(fY[j][:], psx[:])

            # ---------------- Phase C: output chain ----------------
            with (
                tc.tile_pool(name="chain", bufs=1) as chain,
                tc.tile_pool(name="psC", bufs=3, space="PSUM") as psC,
            ):
                u1_sb = chain.tile([P, nb * v], F32R, tag="u1_sb")
                nc.sync.dma_start(
                    u1_sb[:].rearrange("p (t n) -> p t n", n=v),
                    u1ag[:, :].rearrange("(t p) n -> p t n", p=P))
                bt_sb = chain.tile([P, nv * d], F32R, tag="bt_sb")
                nc.sync.dma_start(bt_sb[:], bt_in[:])
                t2 = [chain.tile([P, cw], F32R, tag=f"t2_{vh}", name=f"t2_{vh}")
                      for vh in range(nv)]
                for vh in range(nv):
                    ps2 = psC.tile([P, cw], F32, tag="cps")
                    for k in range(nb):
                        nc.tensor.matmul(
                            ps2[:],
                            u1_sb[:, k * v + vh * P:k * v + (vh + 1) * P],
                            fY[k][:], start=(k == 0), stop=(k == nb - 1))
                    nc.vector.tensor_copy(t2[vh][:], ps2[:])
                for m in range(nb):
                    ps3 = psC.tile([P, cw], F32, tag="cps")
                    for vh in range(nv):
                        nc.tensor.matmul(
                            ps3[:], bt_sb[:, vh * d + m * P:vh * d + (m + 1) * P],
                            t2[vh][:], start=(vh == 0), stop=(vh == nv - 1))
                    ao = chain.tile([P, cw], F32, tag="ao")
                    nc.vector.tensor_tensor(ao[:], fY[m][:].bitcast(F32),
                                            ps3[:], op=sub)
                    nc.sync.dma_start(a_out[:, m * cw:(m + 1) * cw], ao[:])

    nc.compile()
    return nc


_CACHE = {}


def _get_program(cfg: Cfg):
    key = (cfg.d, cfg.v, cfg.ncores, tuple(cfg.newton), cfg.warm)
    if key not in _CACHE:
        _CACHE[key] = build_program(cfg)
    return _CACHE[key]


def run(cfg: Cfg, L, R, B, trace=False):
    global LAST_EXEC_NS
    d, cw, v, nb, kt = cfg.d, cfg.cw, cfg.v, cfg.nb, cfg.kt
    nc = _get_program(cfg)
    L = np.ascontiguousarray(L, np.float32)
    R = np.ascontiguousarray(R, np.float32)
    B = np.ascontiguousarray(B, np.float32)
    LT = np.ascontiguousarray(L.T)

    # lt_t[m, p, t, q] = LT[t*128+p, m*128+q]; h=0 tiles (m < nb) * 0.5
    Y = LT.reshape(kt, P, kt, P)
    lt_t = np.ascontiguousarray(Y.transpose(2, 1, 0, 3))
    lt_t[:nb] *= 0.5
    lt_t = lt_t.reshape(cfg.kdim, cfg.kdim).astype(ml_dtypes.bfloat16)

    SK = 0.5 * (R - R.T)
    b2 = round_f32r(
        (2.0 * B).reshape(nb, P, v).transpose(1, 0, 2).reshape(P, nb * v))
    bt_t = round_f32r(
        np.ascontiguousarray(B.T).reshape(cfg.nv, P, d)
        .transpose(1, 0, 2).reshape(P, cfg.nv * d))
    iden = np.eye(P, dtype=np.float32)
    iden2 = 2.0 * iden
    ones_v = np.ones((P, 1), np.float32)
    ones_r = np.ones((1, P), np.float32)

    in_maps = []
    for c in range(cfg.ncores):
        c0 = c * cw
        rhs1 = LT[:, c0:c0 + cw].reshape(kt, P, cw).transpose(1, 0, 2)
        rhs2 = 0.5 * LT[:, d + c0:d + c0 + cw].reshape(kt, P, cw) \
            .transpose(1, 0, 2)
        rhs = np.concatenate([rhs1, rhs2], axis=2) \
            .reshape(P, kt * 2 * cw).astype(ml_dtypes.bfloat16)
        sk_c = np.ascontiguousarray(
            SK[:, c0:c0 + cw].reshape(nb, P, cw).transpose(1, 0, 2)
            .reshape(P, nb * cw))
        in_maps.append({
            "lt": lt_t,
            "rhs": rhs,
            "b": b2, "bt": bt_t, "sk": sk_c,
            "iden": iden, "iden2": iden2,
            "onesv": ones_v, "onesr": ones_r,
        })
    res = run_bass_kernel_spmd(nc, in_maps, core_ids=list(range(cfg.ncores)),
                               trace=trace)
    LAST_EXEC_NS = res.exec_time_ns
    run.last_results = res.results
    cols = []
    for c in range(cfg.ncores):
        a_t = res.results[c]["a_out"]  # [128, nb*cw]
        cols.append(a_t.reshape(P, nb, cw).transpose(1, 0, 2).reshape(d, cw))
    A = np.concatenate(cols, axis=1)
    return np.ascontiguousarray(A, np.float32)


def kernel(L, R, B, dim_K):
    dim = int(dim_K)
    assert dim == 2048 and L.shape == (4096, 4096)
    cfg = Cfg(d=2048, v=256, ncores=8)
    return run(cfg, L, R, B, trace=False)
